# revision 22
# baseline (speedup 1.0000x reference)
"""Additive (Bahdanau) attention on 8 Trainium2 NeuronCores.

Reference math (per batch b):
    qh = queries @ Wq                  (NQ, H)
    kh = keys    @ Wk                  (NK, H)
    scores[q,k] = sum_h wv[h] * tanh(qh[q,h] + kh[k,h])
    attn = softmax(mask(scores))       mask: k >= valid_len -> -1e6
    out  = attn @ values               (NQ, V)

Sharding: 8 cores = 4 batches x 2 query-halves (128 q-rows each). Each core
owns the full key dimension -> no collectives, host just concatenates.

Per-core device algorithm (NQS=128 q, NK=2048 k, H=32):
  - partitions carry (j, h) = (q mod 4, h)  -> 4*32 = 128 lanes
  - kh4 psum (128, 2048): kh replicated 4x over partition groups, computed by
    4 col-tiled matmuls lhsT=Wk rhs=keys^T
  - qh4 sbuf (128, 32): qh4[(j,h), g] = qh[4g+j, h] via 4 col-tiled matmuls
  - per q-group g (32 groups of 4 q's):
      F_g = tanh(kh4 + bias qh4[:, g])        one ScalarE pass, FD=2048
      scores[4g:4g+4, :] += wv-weighted h-reduction: TensorE matmul with a
      zero-padded (128,128) stationary weight accumulating into scores psum
  - P = exp(scores)  (no max-subtraction needed: |scores| <= ||wv||_1 ~ 5)
  - transpose P via PE, multiply by 0/1 mask column (per-partition scalar)
  - out_unnorm (128, 65) = P_T.T @ [V | 1] accumulated over 16 k-tiles;
    column 64 is the masked softmax denominator l
  - out = out_unnorm[:, :64] * (1/l)

Masked keys contribute exactly 0 (mask multiply) and the missing max
subtraction cancels in the p/l ratio, so this matches the reference exactly
up to fp32 rounding.
"""

import ml_dtypes
import numpy as np

import concourse.bacc as bacc
import concourse.tile as tile
from concourse import mybir
from concourse.bass_utils import run_bass_kernel_spmd

B, NQ, NK = 4, 256, 2048
QKD, H, VD = 64, 32, 64
NQS = 128          # q rows per core
NG = NQS // 4      # 32 q-groups of 4
NKT = NK // 128    # 16 k-tiles
F32 = mybir.dt.float32
BF16 = mybir.dt.bfloat16

_cache = {}


def _build_nc():
    nc = bacc.Bacc("TRN2", debug=False, num_devices=8)

    # blob columns: [0:32]=wq, [32:64]=wk, [64:192]=qTr, [192:320]=ident,
    # [320:336]=maskc  (wq/wk/qTr occupy partitions 0-63)
    d_kT = nc.declare_dram_parameter("kT", [128, NK // 2], BF16, isOutput=False)
    d_blob = nc.declare_dram_parameter("blob", [128, 336], BF16, isOutput=False)
    d_wvb = nc.declare_dram_parameter("wvb", [128, NG * 32], BF16, isOutput=False)
    d_vaug = nc.declare_dram_parameter("vaug", [128, NKT * 65], BF16, isOutput=False)
    d_out = nc.declare_dram_parameter("out", [NQS, VD], F32, isOutput=True)

    TANH = mybir.ActivationFunctionType.Tanh
    EXP = mybir.ActivationFunctionType.Exp

    with tile.TileContext(nc) as tc:
        with (
            tc.tile_pool(name="sb", bufs=1) as sb,
            tc.tile_pool(name="fpool", bufs=2) as fpool,
            tc.tile_pool(name="psA", bufs=1, space="PSUM") as psA,
            tc.tile_pool(name="psB", bufs=1, space="PSUM") as psB,
        ):
            # ---- constant / input tiles ----
            # kT2: [0:64, f] = keys^T[:, f], [64:128, f] = keys^T[:, 1024+f]
            kT_sb = sb.tile([128, NK // 2], BF16, tag="kT")
            blob_sb = sb.tile([128, 336], BF16, tag="blob")
            wvb_sb = sb.tile([128, NG * 32], BF16, tag="wvb")
            vaug_sb = sb.tile([128, NKT * 65], BF16, tag="vaug")
            qh4_sb = sb.tile([128, NG], F32, tag="qh4")
            kh4bf_sb = sb.tile([128, NK], BF16, tag="kh4bf")
            wq_sb = blob_sb[0:QKD, 0:32]
            wk_lo = blob_sb[0:QKD, 32:64]
            wk_hi = blob_sb[QKD:128, 32:64]
            qTr_sb = blob_sb[0:QKD, 64:192]
            ident_sb = blob_sb[:, 192:320]
            maskc_bf = blob_sb[:, 320:336]
            maskc_sb = sb.tile([128, NKT], F32, tag="maskf")
            P_sb = sb.tile([128, NK], BF16, tag="P")
            PT_sb = sb.tile([128, NK], BF16, tag="PT")
            linv_sb = sb.tile([128, 1], F32, tag="linv")
            out_sb = sb.tile([NQS, VD], F32, tag="outsb")

            nc.sync.dma_start(out=kT_sb[:, 0:512], in_=d_kT[:, 0:512])
            nc.gpsimd.dma_start(out=kT_sb[:, 512:1024], in_=d_kT[:, 512:1024])
            nc.scalar.dma_start(out=blob_sb[:], in_=d_blob[:])
            nc.vector.tensor_copy(maskc_sb[:], maskc_bf)

            # ---- psum tiles ----
            kh4c = [psA.tile([128, 512], F32, tag=f"big{c}", name=f"kh4c{c}")
                    for c in range(4)]
            qh4_ps = psB.tile([128, NG], F32, tag="acc")

            # qh4[(j,h), g] = sum_d Wq[d,h] * qTr[d, j*32+g]
            for j in range(4):
                nc.tensor.matmul(
                    qh4_ps[32 * j:32 * (j + 1), :],
                    lhsT=wq_sb,
                    rhs=qTr_sb[:, j * 32:(j + 1) * 32],
                    start=True, stop=True,
                    tile_position=(0, 32 * j),
                )
            nc.vector.tensor_copy(qh4_sb[:], qh4_ps[:])

            # kh4[(j,h), k] = sum_d Wk[d,h] * kT[d,k]  (replicated over j),
            # then narrowed to bf16 in SBUF per 512-chunk
            for c in (0, 2, 1, 3):
                src_rows = 0 if c < 2 else QKD
                rhs = kT_sb[src_rows:src_rows + QKD, (c % 2) * 512:(c % 2 + 1) * 512]
                wk = wk_lo if c < 2 else wk_hi
                for j in range(4):
                    nc.tensor.matmul(
                        kh4c[c][32 * j:32 * (j + 1), :],
                        lhsT=wk,
                        rhs=rhs,
                        start=True, stop=True,
                        tile_position=(src_rows, 32 * j),
                    )
                nc.scalar.copy(
                    kh4bf_sb[:, c * 512:(c + 1) * 512],
                    kh4c[c][:],
                )

            # ---- main loop: DVE bias-add -> one big in-place tanh per chunk
            # -> TensorE h-reduction. Ramped chunk sizes keep startup short.
            scores_ps = psB.tile([128, NK], F32, tag="acc")
            CHUNKS = [2, 2, 4, 8, 8, 4, 2, 1, 1]
            g = 0
            for nch in CHUNKS:
                Fs = fpool.tile([128, nch * NK], BF16, tag=f"Fs{nch}",
                                bufs={1: 4, 2: 4, 4: 3, 8: 2}[nch],
                                name=f"Fs_{g}")
                for half in range(2):
                    for i in range(nch):
                        nc.vector.tensor_scalar_add(
                            Fs[:, i * NK + half * 1024:i * NK + (half + 1) * 1024],
                            kh4bf_sb[:, half * 1024:(half + 1) * 1024],
                            qh4_sb[:, g + i:g + i + 1],
                        )
                nc.scalar.activation(Fs[:], Fs[:], TANH)
                if g == 0:
                    nc.scalar.dma_start(out=wvb_sb[:], in_=d_wvb[:])
                for i in range(nch):
                    gg = g + i
                    G = gg // 8
                    for c in range(4):
                        nc.tensor.matmul(
                            scores_ps[32 * G:32 * (G + 1), c * 512:(c + 1) * 512],
                            lhsT=wvb_sb[:, gg * 32:(gg + 1) * 32],
                            rhs=Fs[:, i * NK + c * 512:i * NK + (c + 1) * 512],
                            start=(gg % 8 == 0), stop=(gg % 8 == 7),
                            skip_group_check=True,
                            tile_position=(0, 32 * G),
                        )
                g += nch

            nc.gpsimd.dma_start(out=vaug_sb[:], in_=d_vaug[:])

            # ---- softmax numerator ----
            nc.scalar.activation(P_sb[:, 0:1024], scores_ps[:, 0:1024], EXP)
            nc.scalar.activation(P_sb[:, 1024:2048], scores_ps[:, 1024:2048], EXP)

            # ---- transpose P (PE) + mask multiply (DVE) + AV matmul ----
            PTb = [psA.tile([128, 1024], BF16, tag=f"big{i}", name=f"PTb{i}")
                   for i in range(4)]
            av_ps = psB.tile([128, 65], F32, tag="acc")
            for t in range(NKT):
                pt = PTb[t % 4][:, (t // 4) * 128:(t // 4 + 1) * 128]
                nc.tensor.transpose(
                    pt,
                    P_sb[:, t * 128:(t + 1) * 128],
                    ident_sb,
                )
                nc.vector.tensor_scalar_mul(
                    PT_sb[:, t * 128:(t + 1) * 128],
                    pt,
                    maskc_sb[:, t:t + 1],
                )
                nc.tensor.matmul(
                    av_ps[:],
                    lhsT=PT_sb[:, t * 128:(t + 1) * 128],
                    rhs=vaug_sb[:, t * 65:(t + 1) * 65],
                    start=(t == 0), stop=(t == NKT - 1),
                )

            # ---- normalize + store ----
            nc.vector.reciprocal(linv_sb[:], av_ps[:, 64:65])
            nc.vector.tensor_scalar_mul(out_sb[:], av_ps[:, 0:64], linv_sb[:])
            nc.sync.dma_start(out=d_out[:], in_=out_sb[:])

    nc.compile()
    return nc


def _host_shards(queries, keys, values, valid_lens, Wq, Wk, wv):
    """Pure data-marshaling: shard, transpose layouts, build mask/weight
    layouts. All FLOPs on the actual tensors happen on device."""
    f32 = np.float32
    queries = np.asarray(queries, f32)
    keys = np.asarray(keys, f32)
    values = np.asarray(values, f32)
    valid_lens = np.asarray(valid_lens)
    Wq = np.asarray(Wq, f32)
    Wk = np.asarray(Wk, f32)
    wv = np.asarray(wv, f32)

    # zero-padded stationary weights for the h-reduction matmuls (M=32
    # supergroup col-tiling: group g writes scores rows 32*(g//8)+4*(g%8)+j)
    wvb = np.zeros((128, NG * 32), f32)
    for g in range(NG):
        for j in range(4):
            wvb[j * 32:(j + 1) * 32, g * 32 + 4 * (g % 8) + j] = wv

    bf16 = ml_dtypes.bfloat16
    blob_base = np.zeros((128, 336), f32)
    blob_base[0:QKD, 0:32] = Wq
    blob_base[0:QKD, 32:64] = Wk
    blob_base[QKD:128, 32:64] = Wk
    blob_base[:, 192:320] = np.eye(128, dtype=f32)
    shared = {"wvb": wvb.astype(bf16)}

    in_maps = []
    for core in range(8):
        b, half = divmod(core, 2)
        qs = queries[b, half * NQS:(half + 1) * NQS]          # (128, 64)
        # qTr[d, j*32+g] = qs[4g+j, d]
        qTr = np.ascontiguousarray(
            qs.T.reshape(QKD, NG, 4).transpose(0, 2, 1)
        ).reshape(QKD, NQS)
        kTf = keys[b].T                                        # (64, 2048)
        kT = np.ascontiguousarray(
            np.concatenate([kTf[:, 0:NK // 2], kTf[:, NK // 2:]], axis=0)
        ).astype(bf16)                                         # (128, 1024)
        v = values[b].reshape(NKT, 128, VD)
        vaug = np.concatenate([v, np.ones((NKT, 128, 1), f32)], axis=2)
        vaug = np.ascontiguousarray(vaug.transpose(1, 0, 2)).reshape(128, NKT * 65).astype(bf16)
        mask = (np.arange(NK) < int(valid_lens[b])).astype(f32)
        blob = blob_base.copy()
        blob[0:QKD, 64:192] = qTr
        blob[:, 320:336] = mask.reshape(NKT, 128).T
        in_maps.append({
            "kT": kT, "blob": blob.astype(bf16), "vaug": vaug, **shared,
        })
    return in_maps


def kernel(queries, keys, values, valid_lens, Wq, Wk, wv, _trace=False):
    if "nc" not in _cache:
        _cache["nc"] = _build_nc()
    nc = _cache["nc"]

    in_maps = _host_shards(queries, keys, values, valid_lens, Wq, Wk, wv)
    res = run_bass_kernel_spmd(nc, in_maps, core_ids=list(range(8)), trace=_trace)
    _cache["last_result"] = res

    out = np.empty((B, NQ, VD), np.float32)
    for core in range(8):
        b, half = divmod(core, 2)
        out[b, half * NQS:(half + 1) * NQS] = res.results[core]["out"]
    return out


# revision 23
# speedup vs baseline: 1.1937x; 1.1937x over previous
"""Additive (Bahdanau) attention on 8 Trainium2 NeuronCores.

Reference math (per batch b):
    qh = queries @ Wq                  (NQ, H)
    kh = keys    @ Wk                  (NK, H)
    scores[q,k] = sum_h wv[h] * tanh(qh[q,h] + kh[k,h])
    attn = softmax(mask(scores))       mask: k >= valid_len -> -1e6
    out  = attn @ values               (NQ, V)

Sharding: 8 cores = 4 batches x 2 query-halves (128 q-rows each). Each core
owns the full key dimension -> no collectives, host just concatenates.

Per-core device algorithm (NQS=128 q, NK=2048 k, H=32):
  - partitions carry (j, h) = (q mod 4, h)  -> 4*32 = 128 lanes
  - kh4 psum (128, 2048): kh replicated 4x over partition groups, computed by
    4 col-tiled matmuls lhsT=Wk rhs=keys^T
  - qh4 sbuf (128, 32): qh4[(j,h), g] = qh[4g+j, h] via 4 col-tiled matmuls
  - per q-group g (32 groups of 4 q's):
      F_g = tanh(kh4 + bias qh4[:, g])        one ScalarE pass, FD=2048
      scores[4g:4g+4, :] += wv-weighted h-reduction: TensorE matmul with a
      zero-padded (128,128) stationary weight accumulating into scores psum
  - P = exp(scores)  (no max-subtraction needed: |scores| <= ||wv||_1 ~ 5)
  - transpose P via PE, multiply by 0/1 mask column (per-partition scalar)
  - out_unnorm (128, 65) = P_T.T @ [V | 1] accumulated over 16 k-tiles;
    column 64 is the masked softmax denominator l
  - out = out_unnorm[:, :64] * (1/l)

Masked keys contribute exactly 0 (mask multiply) and the missing max
subtraction cancels in the p/l ratio, so this matches the reference exactly
up to fp32 rounding.
"""

import ml_dtypes
import numpy as np

import concourse.bacc as bacc
import concourse.tile as tile
from concourse import mybir
from concourse.bass_utils import run_bass_kernel_spmd

B, NQ, NK = 4, 256, 2048
QKD, H, VD = 64, 32, 64
NQS = 128          # q rows per core
NG = NQS // 4      # 32 q-groups of 4
NKT = NK // 128    # 16 k-tiles
F32 = mybir.dt.float32
BF16 = mybir.dt.bfloat16

_cache = {}


def _build_nc():
    nc = bacc.Bacc("TRN2", debug=False, num_devices=8)

    # blob columns: [0:32]=wq, [32:64]=wk, [64:192]=qTr, [192:320]=ident,
    # [320:336]=maskc  (wq/wk/qTr occupy partitions 0-63)
    d_kT = nc.declare_dram_parameter("kT", [128, NK // 2], BF16, isOutput=False)
    d_blob = nc.declare_dram_parameter("blob", [128, 336], BF16, isOutput=False)
    d_wvb = nc.declare_dram_parameter("wvb", [128, NG * 32], BF16, isOutput=False)
    d_vaug = nc.declare_dram_parameter("vaug", [128, NKT * 65], BF16, isOutput=False)
    d_out = nc.declare_dram_parameter("out", [NQS, VD], F32, isOutput=True)

    TANH = mybir.ActivationFunctionType.Tanh
    EXP = mybir.ActivationFunctionType.Exp

    with tile.TileContext(nc) as tc:
        with (
            tc.tile_pool(name="sb", bufs=1) as sb,
            tc.tile_pool(name="fpool", bufs=2) as fpool,
            tc.tile_pool(name="psA", bufs=1, space="PSUM") as psA,
            tc.tile_pool(name="psB", bufs=1, space="PSUM") as psB,
        ):
            # ---- constant / input tiles ----
            # kT2: [0:64, f] = keys^T[:, f], [64:128, f] = keys^T[:, 1024+f]
            kT_sb = sb.tile([128, NK // 2], BF16, tag="kT")
            blob_sb = sb.tile([128, 336], BF16, tag="blob")
            wvb_sb = sb.tile([128, NG * 32], BF16, tag="wvb")
            vaug_sb = sb.tile([128, NKT * 65], BF16, tag="vaug")
            qh4_sb = sb.tile([128, NG], F32, tag="qh4")
            kh4bf_sb = sb.tile([128, NK], BF16, tag="kh4bf")
            wq_sb = blob_sb[0:QKD, 0:32]
            wk_lo = blob_sb[0:QKD, 32:64]
            wk_hi = blob_sb[QKD:128, 32:64]
            qTr_sb = blob_sb[0:QKD, 64:192]
            ident_sb = blob_sb[:, 192:320]
            maskc_bf = blob_sb[:, 320:336]
            maskc_sb = sb.tile([128, NKT], F32, tag="maskf")
            P_sb = sb.tile([128, NK], BF16, tag="P")
            PT_sb = sb.tile([128, NK], BF16, tag="PT")
            linv_sb = sb.tile([128, 1], F32, tag="linv")
            out_sb = sb.tile([NQS, VD], F32, tag="outsb")

            nc.sync.dma_start(out=kT_sb[:, 0:512], in_=d_kT[:, 0:512])
            nc.gpsimd.dma_start(out=kT_sb[:, 512:1024], in_=d_kT[:, 512:1024])
            nc.scalar.dma_start(out=blob_sb[:], in_=d_blob[:])
            nc.vector.tensor_copy(maskc_sb[:], maskc_bf)

            # ---- psum tiles ----
            kh4c = [psA.tile([128, 512], F32, tag=f"big{c}", name=f"kh4c{c}")
                    for c in range(4)]
            qh4_ps = psB.tile([128, NG], F32, tag="acc")

            # qh4[(j,h), g] = sum_d Wq[d,h] * qTr[d, j*32+g]
            for j in range(4):
                nc.tensor.matmul(
                    qh4_ps[32 * j:32 * (j + 1), :],
                    lhsT=wq_sb,
                    rhs=qTr_sb[:, j * 32:(j + 1) * 32],
                    start=True, stop=True,
                    tile_position=(0, 32 * j),
                )
            nc.vector.tensor_copy(qh4_sb[:], qh4_ps[:])

            # kh4[(j,h), k] = sum_d Wk[d,h] * kT[d,k]  (replicated over j),
            # then narrowed to bf16 in SBUF per 512-chunk
            for c in (0, 2, 1, 3):
                src_rows = 0 if c < 2 else QKD
                rhs = kT_sb[src_rows:src_rows + QKD, (c % 2) * 512:(c % 2 + 1) * 512]
                wk = wk_lo if c < 2 else wk_hi
                for j in range(4):
                    nc.tensor.matmul(
                        kh4c[c][32 * j:32 * (j + 1), :],
                        lhsT=wk,
                        rhs=rhs,
                        start=True, stop=True,
                        tile_position=(src_rows, 32 * j),
                    )
                nc.scalar.copy(
                    kh4bf_sb[:, c * 512:(c + 1) * 512],
                    kh4c[c][:],
                )

            # ---- main loop: DVE bias-add -> one big in-place tanh per chunk
            # -> TensorE h-reduction. Ramped chunk sizes keep startup short.
            scores_ps = psB.tile([128, NK], F32, tag="acc")
            CHUNKS = [2, 2, 4, 8, 8, 4, 2, 1, 1]
            g = 0
            for nch in CHUNKS:
                Fs = fpool.tile([128, nch * NK], BF16, tag=f"Fs{nch}",
                                bufs={1: 4, 2: 4, 4: 3, 8: 2}[nch],
                                name=f"Fs_{g}")
                for half in range(2):
                    for i in range(nch):
                        nc.vector.tensor_scalar_add(
                            Fs[:, i * NK + half * 1024:i * NK + (half + 1) * 1024],
                            kh4bf_sb[:, half * 1024:(half + 1) * 1024],
                            qh4_sb[:, g + i:g + i + 1],
                        )
                nc.scalar.activation(Fs[:], Fs[:], TANH)
                if g == 0:
                    nc.scalar.dma_start(out=wvb_sb[:], in_=d_wvb[:])
                for i in range(nch):
                    gg = g + i
                    G = gg // 8
                    for c in range(4):
                        nc.tensor.matmul(
                            scores_ps[32 * G:32 * (G + 1), c * 512:(c + 1) * 512],
                            lhsT=wvb_sb[:, gg * 32:(gg + 1) * 32],
                            rhs=Fs[:, i * NK + c * 512:i * NK + (c + 1) * 512],
                            start=(gg % 8 == 0), stop=(gg % 8 == 7),
                            skip_group_check=True,
                            tile_position=(0, 32 * G),
                        )
                g += nch

            nc.gpsimd.dma_start(out=vaug_sb[:], in_=d_vaug[:])

            # ---- softmax numerator ----
            nc.scalar.activation(P_sb[:, 0:1024], scores_ps[:, 0:1024], EXP)
            nc.scalar.activation(P_sb[:, 1024:2048], scores_ps[:, 1024:2048], EXP)

            # ---- transpose P (PE) + mask multiply (DVE) + AV matmul ----
            PTb = [psA.tile([128, 1024], BF16, tag=f"big{i}", name=f"PTb{i}")
                   for i in range(4)]
            av_ps = psB.tile([128, 65], F32, tag="acc")
            for t in range(NKT):
                pt = PTb[t % 4][:, (t // 4) * 128:(t // 4 + 1) * 128]
                nc.tensor.transpose(
                    pt,
                    P_sb[:, t * 128:(t + 1) * 128],
                    ident_sb,
                )
                nc.vector.tensor_scalar_mul(
                    PT_sb[:, t * 128:(t + 1) * 128],
                    pt,
                    maskc_sb[:, t:t + 1],
                )
                nc.tensor.matmul(
                    av_ps[:],
                    lhsT=PT_sb[:, t * 128:(t + 1) * 128],
                    rhs=vaug_sb[:, t * 65:(t + 1) * 65],
                    start=(t == 0), stop=(t == NKT - 1),
                )

            # ---- normalize + store ----
            nc.vector.reciprocal(linv_sb[:], av_ps[:, 64:65])
            nc.vector.tensor_scalar_mul(out_sb[:], av_ps[:, 0:64], linv_sb[:])
            nc.sync.dma_start(out=d_out[:], in_=out_sb[:])

    nc.compile()
    return nc


def _host_shards(queries, keys, values, valid_lens, Wq, Wk, wv):
    """Pure data-marshaling: shard, transpose layouts, build mask/weight
    layouts. All FLOPs on the actual tensors happen on device."""
    f32 = np.float32
    queries = np.asarray(queries, f32)
    keys = np.asarray(keys, f32)
    values = np.asarray(values, f32)
    valid_lens = np.asarray(valid_lens)
    Wq = np.asarray(Wq, f32)
    Wk = np.asarray(Wk, f32)
    wv = np.asarray(wv, f32)

    # zero-padded stationary weights for the h-reduction matmuls (M=32
    # supergroup col-tiling: group g writes scores rows 32*(g//8)+4*(g%8)+j)
    wvb = np.zeros((128, NG * 32), f32)
    for g in range(NG):
        for j in range(4):
            wvb[j * 32:(j + 1) * 32, g * 32 + 4 * (g % 8) + j] = wv

    bf16 = ml_dtypes.bfloat16
    blob_base = np.zeros((128, 336), f32)
    blob_base[0:QKD, 0:32] = Wq
    blob_base[0:QKD, 32:64] = Wk
    blob_base[QKD:128, 32:64] = Wk
    blob_base[:, 192:320] = np.eye(128, dtype=f32)
    shared = {"wvb": wvb.astype(bf16)}

    in_maps = []
    for core in range(8):
        b, half = divmod(core, 2)
        qs = queries[b, half * NQS:(half + 1) * NQS]          # (128, 64)
        # qTr[d, j*32+g] = qs[4g+j, d]
        qTr = np.ascontiguousarray(
            qs.T.reshape(QKD, NG, 4).transpose(0, 2, 1)
        ).reshape(QKD, NQS)
        kTf = keys[b].T                                        # (64, 2048)
        kT = np.ascontiguousarray(
            np.concatenate([kTf[:, 0:NK // 2], kTf[:, NK // 2:]], axis=0)
        ).astype(bf16)                                         # (128, 1024)
        v = values[b].reshape(NKT, 128, VD)
        vaug = np.concatenate([v, np.ones((NKT, 128, 1), f32)], axis=2)
        vaug = np.ascontiguousarray(vaug.transpose(1, 0, 2)).reshape(128, NKT * 65).astype(bf16)
        mask = (np.arange(NK) < int(valid_lens[b])).astype(f32)
        blob = blob_base.copy()
        blob[0:QKD, 64:192] = qTr
        blob[:, 320:336] = mask.reshape(NKT, 128).T
        in_maps.append({
            "kT": kT, "blob": blob.astype(bf16), "vaug": vaug, **shared,
        })
    return in_maps


def kernel(queries, keys, values, valid_lens, Wq, Wk, wv, _trace=False):
    if "nc" not in _cache:
        _cache["nc"] = _build_nc()
    nc = _cache["nc"]

    in_maps = _host_shards(queries, keys, values, valid_lens, Wq, Wk, wv)
    res = None
    for attempt in range(3):
        try:
            res = run_bass_kernel_spmd(
                nc, in_maps, core_ids=list(range(8)), trace=_trace
            )
            break
        except Exception:
            # transient NRT/device errors occasionally surface on first
            # launch; retry with a freshly compiled graph on the last try
            if attempt == 2:
                raise
            if attempt == 1:
                _cache.pop("nc", None)
                _cache["nc"] = nc = _build_nc()
    _cache["last_result"] = res

    out = np.empty((B, NQ, VD), np.float32)
    for core in range(8):
        b, half = divmod(core, 2)
        out[b, half * NQS:(half + 1) * NQS] = res.results[core]["out"]
    return out


# revision 29
# speedup vs baseline: 1.1949x; 1.0010x over previous
"""Additive (Bahdanau) attention on 8 Trainium2 NeuronCores.

Reference math (per batch b):
    qh = queries @ Wq                  (NQ, H)
    kh = keys    @ Wk                  (NK, H)
    scores[q,k] = sum_h wv[h] * tanh(qh[q,h] + kh[k,h])
    attn = softmax(mask(scores))       mask: k >= valid_len -> -1e6
    out  = attn @ values               (NQ, V)

Sharding: 8 cores = 4 batches x 2 query-halves (128 q-rows each). Each core
owns the full key dimension -> no collectives, host just concatenates.

Per-core device algorithm (NQS=128 q, NK=2048 k, H=32):
  - partitions carry (j, h) = (q mod 4, h)  -> 4*32 = 128 lanes
  - kh4 (kh replicated 4x over partition groups) via col-tiled matmuls
    lhsT=Wk rhs=keys^T into 4 per-bank psum tiles, narrowed to one bf16
    sbuf tensor by ScalarE/DVE copies
  - qh4 sbuf (128, 32): qh4[(j,h), g] = qh[4g+j, h] via 4 col-tiled matmuls
  - main loop over q-group chunks (sizes 1,1,2,4,8,8,4,2,1,1 over the 32
    groups of 4 q's; ramped so the pipeline starts early and the final
    scores land early):
      DVE: Fs[:, i*2048:...] = kh4_bf16 + qh4[:, g+i]  (per-partition
           scalar add, 4x perf mode)
      ScalarE: one big in-place tanh over the whole chunk (FD up to 16K,
           amortizes the ~300ns per-instruction overhead; ScalarE is the
           global bottleneck at ~55us of pure tanh work per core)
      TensorE: per group, 4 matmuls (N=512) with a zero-padded (128, 32)
           stationary weight (M=32 supergroup col-tiling) accumulating
           scores into psum
  - P = exp(scores) straight from psum (no max-subtraction needed:
    |scores| <= ||wv||_1 ~ 5, and the shift cancels in the p/l ratio)
  - transpose P via PE (bank-rotated psum tiles), multiply by the 0/1 mask
    column (per-partition scalar) on DVE
  - out_unnorm (128, 65) = P_T.T @ [V | 1] accumulated over 16 k-tiles;
    column 64 is the masked softmax denominator l
  - out = out_unnorm[:, :64] * (1/l)

Masked keys contribute exactly 0 (mask multiply) and the missing max
subtraction cancels in the p/l ratio, so this matches the reference exactly
up to rounding; bf16 matmul inputs (fp32 psum accumulation everywhere)
give ~3.5e-3 relative error on the final output.
"""

import ml_dtypes
import numpy as np

import concourse.bacc as bacc
import concourse.tile as tile
from concourse import mybir
from concourse.bass_utils import run_bass_kernel_spmd

B, NQ, NK = 4, 256, 2048
QKD, H, VD = 64, 32, 64
NQS = 128          # q rows per core
NG = NQS // 4      # 32 q-groups of 4
NKT = NK // 128    # 16 k-tiles
F32 = mybir.dt.float32
BF16 = mybir.dt.bfloat16

_cache = {}


def _build_nc():
    nc = bacc.Bacc("TRN2", debug=False, num_devices=8, monotonic_sem_count=0, enable_asserts=False)

    # blob columns: [0:32]=wq, [32:64]=wk, [64:192]=qTr, [192:320]=ident,
    # [320:336]=maskc  (wq/wk/qTr occupy partitions 0-63)
    d_kT = nc.declare_dram_parameter("kT", [128, NK // 2], BF16, isOutput=False)
    d_blob = nc.declare_dram_parameter("blob", [128, 336], BF16, isOutput=False)
    d_wvb = nc.declare_dram_parameter("wvb", [128, NG * 32], BF16, isOutput=False)
    d_vaug = nc.declare_dram_parameter("vaug", [128, NKT * 65], BF16, isOutput=False)
    d_out = nc.declare_dram_parameter("out", [NQS, VD], F32, isOutput=True)

    TANH = mybir.ActivationFunctionType.Tanh
    EXP = mybir.ActivationFunctionType.Exp

    with tile.TileContext(nc) as tc:
        with (
            tc.tile_pool(name="sb", bufs=1) as sb,
            tc.tile_pool(name="fpool", bufs=2) as fpool,
            tc.tile_pool(name="psA", bufs=1, space="PSUM") as psA,
            tc.tile_pool(name="psB", bufs=1, space="PSUM") as psB,
        ):
            # ---- constant / input tiles ----
            # kT2: [0:64, f] = keys^T[:, f], [64:128, f] = keys^T[:, 1024+f]
            kT_sb = sb.tile([128, NK // 2], BF16, tag="kT")
            blob_sb = sb.tile([128, 336], BF16, tag="blob")
            wvb_sb = sb.tile([128, NG * 32], BF16, tag="wvb")
            vaug_sb = sb.tile([128, NKT * 65], BF16, tag="vaug")
            qh4_sb = sb.tile([128, NG], F32, tag="qh4")
            kh4bf_sb = sb.tile([128, NK], BF16, tag="kh4bf")
            wq_sb = blob_sb[0:QKD, 0:32]
            wk_lo = blob_sb[0:QKD, 32:64]
            wk_hi = blob_sb[QKD:128, 32:64]
            qTr_sb = blob_sb[0:QKD, 64:192]
            ident_sb = blob_sb[:, 192:320]
            maskc_bf = blob_sb[:, 320:336]
            maskc_sb = sb.tile([128, NKT], F32, tag="maskf")
            P_sb = sb.tile([128, NK], BF16, tag="P")
            PT_sb = sb.tile([128, NK], BF16, tag="PT")
            linv_sb = sb.tile([128, 1], F32, tag="linv")
            out_sb = sb.tile([NQS, VD], F32, tag="outsb")

            nc.sync.dma_start(out=kT_sb[:, 0:512], in_=d_kT[:, 0:512])
            nc.gpsimd.dma_start(out=kT_sb[:, 512:1024], in_=d_kT[:, 512:1024])
            nc.scalar.dma_start(out=blob_sb[:], in_=d_blob[:])
            nc.vector.tensor_copy(maskc_sb[:], maskc_bf)

            # ---- psum tiles ----
            kh4c = [psA.tile([128, 512], F32, tag=f"big{c}", name=f"kh4c{c}")
                    for c in range(4)]
            qh4_ps = psB.tile([128, NG], F32, tag="acc")

            # qh4[(j,h), g] = sum_d Wq[d,h] * qTr[d, j*32+g]
            for j in range(4):
                nc.tensor.matmul(
                    qh4_ps[32 * j:32 * (j + 1), :],
                    lhsT=wq_sb,
                    rhs=qTr_sb[:, j * 32:(j + 1) * 32],
                    start=True, stop=True,
                    tile_position=(0, 32 * j),
                )
            nc.vector.tensor_copy(qh4_sb[:], qh4_ps[:])

            # kh4[(j,h), k] = sum_d Wk[d,h] * kT[d,k]  (replicated over j),
            # then narrowed to bf16 in SBUF per 512-chunk
            for c in (0, 2, 1, 3):
                src_rows = 0 if c < 2 else QKD
                rhs = kT_sb[src_rows:src_rows + QKD, (c % 2) * 512:(c % 2 + 1) * 512]
                wk = wk_lo if c < 2 else wk_hi
                for j in range(4):
                    nc.tensor.matmul(
                        kh4c[c][32 * j:32 * (j + 1), :],
                        lhsT=wk,
                        rhs=rhs,
                        start=True, stop=True,
                        tile_position=(src_rows, 32 * j),
                    )
                copy_eng = nc.scalar.copy if c < 2 else nc.vector.tensor_copy
                copy_eng(
                    kh4bf_sb[:, c * 512:(c + 1) * 512],
                    kh4c[c][:],
                )

            # ---- main loop: DVE bias-add -> one big in-place tanh per chunk
            # -> TensorE h-reduction. Ramped chunk sizes keep startup short.
            scores_ps = psB.tile([128, NK], F32, tag="acc")
            CHUNKS = [1, 1, 2, 4, 8, 8, 4, 2, 1, 1]
            g = 0
            for nch in CHUNKS:
                Fs = fpool.tile([128, nch * NK], BF16, tag=f"Fs{nch}",
                                bufs={1: 4, 2: 4, 4: 3, 8: 2}[nch],
                                name=f"Fs_{g}")
                for half in range(2):
                    for i in range(nch):
                        nc.vector.tensor_scalar_add(
                            Fs[:, i * NK + half * 1024:i * NK + (half + 1) * 1024],
                            kh4bf_sb[:, half * 1024:(half + 1) * 1024],
                            qh4_sb[:, g + i:g + i + 1],
                        )
                nc.scalar.activation(Fs[:], Fs[:], TANH)
                if g == 0:
                    nc.scalar.dma_start(out=wvb_sb[:], in_=d_wvb[:])
                for i in range(nch):
                    gg = g + i
                    G = gg // 8
                    for c in range(4):
                        nc.tensor.matmul(
                            scores_ps[32 * G:32 * (G + 1), c * 512:(c + 1) * 512],
                            lhsT=wvb_sb[:, gg * 32:(gg + 1) * 32],
                            rhs=Fs[:, i * NK + c * 512:i * NK + (c + 1) * 512],
                            start=(gg % 8 == 0), stop=(gg % 8 == 7),
                            skip_group_check=True,
                            tile_position=(0, 32 * G),
                        )
                g += nch

            nc.gpsimd.dma_start(out=vaug_sb[:], in_=d_vaug[:])

            # ---- softmax numerator ----
            nc.scalar.activation(P_sb[:, 0:1024], scores_ps[:, 0:1024], EXP)
            nc.scalar.activation(P_sb[:, 1024:2048], scores_ps[:, 1024:2048], EXP)

            # ---- transpose P (PE) + mask multiply (DVE) + AV matmul ----
            PTb = [psA.tile([128, 1024], BF16, tag=f"big{i}", name=f"PTb{i}")
                   for i in range(4)]
            av_ps = psB.tile([128, 65], F32, tag="acc")
            for t in range(NKT):
                pt = PTb[t % 4][:, (t // 4) * 128:(t // 4 + 1) * 128]
                nc.tensor.transpose(
                    pt,
                    P_sb[:, t * 128:(t + 1) * 128],
                    ident_sb,
                )
                nc.vector.tensor_scalar_mul(
                    PT_sb[:, t * 128:(t + 1) * 128],
                    pt,
                    maskc_sb[:, t:t + 1],
                )
                nc.tensor.matmul(
                    av_ps[:],
                    lhsT=PT_sb[:, t * 128:(t + 1) * 128],
                    rhs=vaug_sb[:, t * 65:(t + 1) * 65],
                    start=(t == 0), stop=(t == NKT - 1),
                )

            # ---- normalize + store ----
            nc.vector.reciprocal(linv_sb[:], av_ps[:, 64:65])
            nc.vector.tensor_scalar_mul(out_sb[:], av_ps[:, 0:64], linv_sb[:])
            nc.sync.dma_start(out=d_out[:], in_=out_sb[:])

    nc.compile()
    return nc


def _host_shards(queries, keys, values, valid_lens, Wq, Wk, wv):
    """Pure data-marshaling: shard, transpose layouts, build mask/weight
    layouts. All FLOPs on the actual tensors happen on device."""
    f32 = np.float32
    queries = np.asarray(queries, f32)
    keys = np.asarray(keys, f32)
    values = np.asarray(values, f32)
    valid_lens = np.asarray(valid_lens)
    Wq = np.asarray(Wq, f32)
    Wk = np.asarray(Wk, f32)
    wv = np.asarray(wv, f32)

    # zero-padded stationary weights for the h-reduction matmuls (M=32
    # supergroup col-tiling: group g writes scores rows 32*(g//8)+4*(g%8)+j)
    wvb = np.zeros((128, NG * 32), f32)
    for g in range(NG):
        for j in range(4):
            wvb[j * 32:(j + 1) * 32, g * 32 + 4 * (g % 8) + j] = wv

    bf16 = ml_dtypes.bfloat16
    blob_base = np.zeros((128, 336), f32)
    blob_base[0:QKD, 0:32] = Wq
    blob_base[0:QKD, 32:64] = Wk
    blob_base[QKD:128, 32:64] = Wk
    blob_base[:, 192:320] = np.eye(128, dtype=f32)
    shared = {"wvb": wvb.astype(bf16)}

    in_maps = []
    for core in range(8):
        b, half = divmod(core, 2)
        qs = queries[b, half * NQS:(half + 1) * NQS]          # (128, 64)
        # qTr[d, j*32+g] = qs[4g+j, d]
        qTr = np.ascontiguousarray(
            qs.T.reshape(QKD, NG, 4).transpose(0, 2, 1)
        ).reshape(QKD, NQS)
        kTf = keys[b].T                                        # (64, 2048)
        kT = np.ascontiguousarray(
            np.concatenate([kTf[:, 0:NK // 2], kTf[:, NK // 2:]], axis=0)
        ).astype(bf16)                                         # (128, 1024)
        v = values[b].reshape(NKT, 128, VD)
        vaug = np.concatenate([v, np.ones((NKT, 128, 1), f32)], axis=2)
        vaug = np.ascontiguousarray(vaug.transpose(1, 0, 2)).reshape(128, NKT * 65).astype(bf16)
        mask = (np.arange(NK) < int(valid_lens[b])).astype(f32)
        blob = blob_base.copy()
        blob[0:QKD, 64:192] = qTr
        blob[:, 320:336] = mask.reshape(NKT, 128).T
        in_maps.append({
            "kT": kT, "blob": blob.astype(bf16), "vaug": vaug, **shared,
        })
    return in_maps


def kernel(queries, keys, values, valid_lens, Wq, Wk, wv, _trace=False):
    if "nc" not in _cache:
        _cache["nc"] = _build_nc()
    nc = _cache["nc"]

    in_maps = _host_shards(queries, keys, values, valid_lens, Wq, Wk, wv)
    res = None
    for attempt in range(3):
        try:
            res = run_bass_kernel_spmd(
                nc, in_maps, core_ids=list(range(8)), trace=_trace
            )
            break
        except Exception:
            # transient NRT/device errors occasionally surface on first
            # launch; retry with a freshly compiled graph on the last try
            if attempt == 2:
                raise
            if attempt == 1:
                _cache.pop("nc", None)
                _cache["nc"] = nc = _build_nc()
    _cache["last_result"] = res

    out = np.empty((B, NQ, VD), np.float32)
    for core in range(8):
        b, half = divmod(core, 2)
        out[b, half * NQS:(half + 1) * NQS] = res.results[core]["out"]
    return out


# revision 31
# speedup vs baseline: 1.8235x; 1.5261x over previous
"""Additive (Bahdanau) attention on 8 Trainium2 NeuronCores.

Reference math (per batch b):
    qh = queries @ Wq                  (NQ, H)
    kh = keys    @ Wk                  (NK, H)
    scores[q,k] = sum_h wv[h] * tanh(qh[q,h] + kh[k,h])
    attn = softmax(mask(scores))       mask: k >= valid_len -> -1e6
    out  = attn @ values               (NQ, V)

Sharding (flash-style, valid-length aware): masked keys contribute exactly
zero to the softmax (the reference's exp(-1e6 - max) underflows to 0.0), so
only k < valid_len needs computing. The valid (batch, q-half, k-chunk) space
is split into uniform work tiles of (128 q-rows x 512 keys); tiles are
distributed round-robin over the 8 cores (padded with zero-mask dummy tiles
to a multiple of 8, T = tiles-per-core is 1..4). Every core runs the same
SPMD graph over T tiles. Each tile emits the UNNORMALIZED partials
(sum_k p*V | sum_k p) as a (128, 65) block; the host sums partials of the
same (batch, q-half) across tiles and divides - the cross-shard softmax
renormalization. No max-subtraction is needed: |scores| <= ||wv||_1 (~5),
so exp never overflows, and the missing shift cancels in the p/l ratio.
Math is exact up to rounding; bf16 matmul inputs with fp32 PSUM
accumulation give ~3e-3 relative error on the final output.

Per-tile device pipeline (q=128 -> 32 groups of 4, k=512):
  - partitions carry (j, h) = (q mod 4, h) -> 4*32 = 128 lanes
  - kh4 (kh replicated 4x over partition groups) via one col-tiled matmul
    set into a 1-bank psum tile, narrowed to bf16 in SBUF
  - qh4[(j,h), g] = qh[4g+j, h] via 4 col-tiled matmuls
  - loop over q-group chunks (2,2,4,8,8,8): DVE adds the per-group bias
    (per-partition scalar), ScalarE runs one big in-place tanh per chunk,
    TensorE reduces over h with zero-padded (128, 32) stationary weights
    (M=32 supergroup col-tiling) accumulating scores in psum
  - P = exp(scores) from psum; PE transposes P (4 tiles of 128x128);
    DVE multiplies by the 0/1 mask column during the psum->sbuf copy;
    accumulate [V | 1] matmuls into the (128, 65) partial output
Successive tiles pipeline: tile t+1's tanh stream runs while tile t's
softmax tail finishes, so only the last tile's tail is exposed.
"""

import ml_dtypes
import numpy as np

import concourse.bacc as bacc
import concourse.tile as tile
from concourse import mybir
from concourse.bass_utils import run_bass_kernel_spmd

B, NQ, NK = 4, 256, 2048
QKD, H, VD = 64, 32, 64
NQS = 128          # q rows per tile
NG = NQS // 4      # 32 q-groups of 4
KC = 512           # keys per tile
KT = KC // 128     # 4 k-subtiles per tile
CHUNKS = [2, 2, 4, 8, 8, 8]
F32 = mybir.dt.float32
BF16 = mybir.dt.bfloat16

_cache = {}


def _build_nc(T):
    """Build the SPMD graph processing T work tiles per core."""
    nc = bacc.Bacc("TRN2", debug=False, num_devices=8,
                   monotonic_sem_count=0, enable_asserts=False)

    # blob columns: [0:32]=wq, [32:64]=wk, [64:64+128T]=qTr per tile,
    # [64+128T:192+128T]=ident, then 4T mask columns
    BW = 192 + 132 * T
    d_kT = nc.declare_dram_parameter("kT", [QKD, KC * T], BF16, isOutput=False)
    d_blob = nc.declare_dram_parameter("blob", [128, BW], BF16, isOutput=False)
    d_wvb = nc.declare_dram_parameter("wvb", [128, NG * 32], BF16, isOutput=False)
    d_vaug = nc.declare_dram_parameter("vaug", [128, KT * 65 * T], BF16,
                                       isOutput=False)
    d_out = nc.declare_dram_parameter("out", [NQS, 65 * T], F32, isOutput=True)

    TANH = mybir.ActivationFunctionType.Tanh
    EXP = mybir.ActivationFunctionType.Exp

    with tile.TileContext(nc) as tc:
        with (
            tc.tile_pool(name="sb", bufs=1) as sb,
            tc.tile_pool(name="fpool", bufs=2) as fpool,
            tc.tile_pool(name="psA", bufs=1, space="PSUM") as psA,
            tc.tile_pool(name="psB", bufs=1, space="PSUM") as psB,
        ):
            kT_sb = sb.tile([QKD, KC * T], BF16, tag="kT")
            blob_sb = sb.tile([128, BW], BF16, tag="blob")
            wvb_sb = sb.tile([128, NG * 32], BF16, tag="wvb")
            vaug_sb = sb.tile([128, KT * 65 * T], BF16, tag="vaug")
            qh4_sb = sb.tile([128, NG * T], F32, tag="qh4")
            kh4bf_sb = sb.tile([128, KC * T], BF16, tag="kh4bf")
            wq_sb = blob_sb[0:QKD, 0:32]
            wk_sb = blob_sb[0:QKD, 32:64]
            qTr_all = blob_sb[0:QKD, 64:64 + 128 * T]
            ident_sb = blob_sb[:, 64 + 128 * T:192 + 128 * T]
            maskc_bf = blob_sb[:, 192 + 128 * T:BW]
            maskc_sb = sb.tile([128, 4 * T], F32, tag="maskf")
            P_sb = sb.tile([128, KC * T], BF16, tag="P")
            PT_sb = sb.tile([128, KC * T], BF16, tag="PT")
            out_sb = sb.tile([NQS, 65 * T], F32, tag="outsb")

            # split the early DMAs across engine queues
            nc.sync.dma_start(out=kT_sb[:, 0:KC], in_=d_kT[:, 0:KC])
            if T > 1:
                nc.scalar.dma_start(out=kT_sb[:, KC:], in_=d_kT[:, KC:])
            nc.gpsimd.dma_start(out=blob_sb[:], in_=d_blob[:])
            nc.vector.tensor_copy(maskc_sb[:], maskc_bf)

            qh4_ps = psB.tile([128, NG * T], F32, tag="sc0")
            for t in range(T):
                for j in range(4):
                    nc.tensor.matmul(
                        qh4_ps[32 * j:32 * (j + 1), t * NG:(t + 1) * NG],
                        lhsT=wq_sb,
                        rhs=qTr_all[:, t * 128 + j * 32:t * 128 + (j + 1) * 32],
                        start=True, stop=True,
                        tile_position=(0, 32 * j),
                    )
            nc.vector.tensor_copy(qh4_sb[:], qh4_ps[:])

            # per-tile kh4 psum (1 bank each) -> bf16 sbuf
            kh4c = [psA.tile([128, KC], F32, tag=f"kh{t}", name=f"kh4c{t}")
                    for t in range(T)]
            for t in range(T):
                for j in range(4):
                    nc.tensor.matmul(
                        kh4c[t][32 * j:32 * (j + 1), :],
                        lhsT=wk_sb,
                        rhs=kT_sb[:, t * KC:(t + 1) * KC],
                        start=True, stop=True,
                        tile_position=(0, 32 * j),
                    )
                # alternate cast engines so they pipeline
                cp = nc.scalar.copy if t % 2 == 0 else nc.vector.tensor_copy
                cp(kh4bf_sb[:, t * KC:(t + 1) * KC], kh4c[t][:])

            scores = [psB.tile([128, KC], F32, tag=f"sc{t}", name=f"sc{t}")
                      for t in range(T)]
            PTb = [psA.tile([128, KC], BF16, tag=f"kh{t}", name=f"PTb{t}")
                   for t in range(T)]
            av = [psB.tile([128, 65], F32, tag=f"sc{t}", name=f"av{t}")
                  for t in range(T)]

            for t in range(T):
                # ---- tanh + h-reduction over this tile's 512 keys ----
                g = 0
                for nch in CHUNKS:
                    Fs = fpool.tile([128, nch * KC], BF16, tag=f"Fs{nch}",
                                    bufs={2: 4, 4: 3, 8: 3}[nch],
                                    name=f"Fs_{t}_{g}")
                    for i in range(nch):
                        nc.vector.tensor_scalar_add(
                            Fs[:, i * KC:(i + 1) * KC],
                            kh4bf_sb[:, t * KC:(t + 1) * KC],
                            qh4_sb[:, t * NG + g + i:t * NG + g + i + 1],
                        )
                    nc.scalar.activation(Fs[:], Fs[:], TANH)
                    if t == 0 and g == 0:
                        nc.scalar.dma_start(out=wvb_sb[:], in_=d_wvb[:])
                        nc.gpsimd.dma_start(out=vaug_sb[:], in_=d_vaug[:])
                    for i in range(nch):
                        gg = g + i
                        G = gg // 8
                        nc.tensor.matmul(
                            scores[t][32 * G:32 * (G + 1), :],
                            lhsT=wvb_sb[:, gg * 32:(gg + 1) * 32],
                            rhs=Fs[:, i * KC:(i + 1) * KC],
                            start=(gg % 8 == 0), stop=(gg % 8 == 7),
                            skip_group_check=True,
                            tile_position=(0, 32 * G),
                        )
                    g += nch

                # ---- softmax numerator + masked AV partials ----
                nc.scalar.activation(
                    P_sb[:, t * KC:(t + 1) * KC], scores[t][:], EXP)
                for s in range(KT):
                    pcol = t * KC + s * 128
                    pt = PTb[t][:, s * 128:(s + 1) * 128]
                    nc.tensor.transpose(
                        pt, P_sb[:, pcol:pcol + 128], ident_sb)
                    nc.vector.tensor_scalar_mul(
                        PT_sb[:, pcol:pcol + 128], pt,
                        maskc_sb[:, t * KT + s:t * KT + s + 1],
                    )
                    nc.tensor.matmul(
                        av[t][:],
                        lhsT=PT_sb[:, pcol:pcol + 128],
                        rhs=vaug_sb[:, (t * KT + s) * 65:(t * KT + s + 1) * 65],
                        start=(s == 0), stop=(s == KT - 1),
                    )
                nc.vector.tensor_copy(
                    out_sb[:, t * 65:(t + 1) * 65], av[t][:])

            nc.sync.dma_start(out=d_out[:], in_=out_sb[:])

    nc.compile()
    return nc


def _host_shards(queries, keys, values, valid_lens, Wq, Wk, wv):
    """Build the balanced valid-key tile assignment and per-core inputs.
    Host work is layout/marshaling only; all tensor FLOPs run on device."""
    f32 = np.float32
    bf16 = ml_dtypes.bfloat16
    queries = np.asarray(queries, f32)
    keys = np.asarray(keys, f32)
    values = np.asarray(values, f32)
    valid_lens = np.asarray(valid_lens)
    Wq = np.asarray(Wq, f32)
    Wk = np.asarray(Wk, f32)
    wv = np.asarray(wv, f32)

    # work tiles: (batch, q-half, k-chunk) over the valid key range
    tiles = []
    for b in range(B):
        nk_chunks = max(1, int(np.ceil(int(valid_lens[b]) / KC)))
        for half in range(NQ // NQS):
            for kc in range(nk_chunks):
                tiles.append((b, half, kc))
    while len(tiles) % 8 != 0:
        tiles.append(None)                     # zero-mask dummy
    T = len(tiles) // 8

    # zero-padded stationary weights (M=32 supergroup col-tiling)
    wvb = np.zeros((128, NG * 32), f32)
    for g in range(NG):
        for j in range(4):
            wvb[j * 32:(j + 1) * 32, g * 32 + 4 * (g % 8) + j] = wv

    BW = 192 + 132 * T
    blob_base = np.zeros((128, BW), f32)
    blob_base[0:QKD, 0:32] = Wq
    blob_base[0:QKD, 32:64] = Wk
    blob_base[:, 64 + 128 * T:192 + 128 * T] = np.eye(128, dtype=f32)
    shared = {"wvb": wvb.astype(bf16)}

    assign = [tiles[c::8] for c in range(8)]   # round-robin -> balanced
    in_maps = []
    for core in range(8):
        kT = np.zeros((QKD, KC * T), f32)
        vaug = np.zeros((128, KT * 65 * T), f32)
        blob = blob_base.copy()
        for t, tl in enumerate(assign[core]):
            if tl is None:
                continue
            b, half, kc = tl
            qs = queries[b, half * NQS:(half + 1) * NQS]      # (128, 64)
            qTr = np.ascontiguousarray(
                qs.T.reshape(QKD, NG, 4).transpose(0, 2, 1)).reshape(QKD, NQS)
            blob[0:QKD, 64 + 128 * t:64 + 128 * (t + 1)] = qTr
            kT[:, t * KC:(t + 1) * KC] = keys[b, kc * KC:(kc + 1) * KC].T
            v = values[b, kc * KC:(kc + 1) * KC].reshape(KT, 128, VD)
            va = np.concatenate([v, np.ones((KT, 128, 1), f32)], axis=2)
            vaug[:, t * KT * 65:(t + 1) * KT * 65] = (
                va.transpose(1, 0, 2).reshape(128, KT * 65))
            kmask = (np.arange(kc * KC, (kc + 1) * KC)
                     < int(valid_lens[b])).astype(f32)
            blob[:, 192 + 128 * T + 4 * t:192 + 128 * T + 4 * (t + 1)] = (
                kmask.reshape(KT, 128).T)
        in_maps.append({
            "kT": np.ascontiguousarray(kT).astype(bf16),
            "blob": blob.astype(bf16),
            "vaug": np.ascontiguousarray(vaug).astype(bf16),
            **shared,
        })
    return T, assign, in_maps


def kernel(queries, keys, values, valid_lens, Wq, Wk, wv, _trace=False):
    T, assign, in_maps = _host_shards(
        queries, keys, values, valid_lens, Wq, Wk, wv)
    if ("nc", T) not in _cache:
        _cache[("nc", T)] = _build_nc(T)
    nc = _cache[("nc", T)]

    res = None
    for attempt in range(3):
        try:
            res = run_bass_kernel_spmd(
                nc, in_maps, core_ids=list(range(8)), trace=_trace
            )
            break
        except Exception:
            if attempt == 2:
                raise
            if attempt == 1:
                _cache.pop(("nc", T), None)
                _cache[("nc", T)] = nc = _build_nc(T)
    _cache["last_result"] = res

    # cross-shard softmax renormalization (the unshard/combine step)
    acc = np.zeros((B, NQ // NQS, NQS, VD + 1), np.float64)
    for core in range(8):
        part = res.results[core]["out"]        # (128, 65*T)
        for t, tl in enumerate(assign[core]):
            if tl is None:
                continue
            b, half, _ = tl
            acc[b, half] += part[:, t * 65:(t + 1) * 65].astype(np.float64)
    out = acc[..., :VD] / acc[..., VD:VD + 1]
    return np.ascontiguousarray(
        out.reshape(B, NQ, VD).astype(np.float32))


# revision 37
# speedup vs baseline: 1.8701x; 1.0256x over previous
"""Additive (Bahdanau) attention on 8 Trainium2 NeuronCores.

Reference math (per batch b):
    qh = queries @ Wq                  (NQ, H)
    kh = keys    @ Wk                  (NK, H)
    scores[q,k] = sum_h wv[h] * tanh(qh[q,h] + kh[k,h])
    attn = softmax(mask(scores))       mask: k >= valid_len -> -1e6
    out  = attn @ values               (NQ, V)

Sharding (flash-style, valid-length aware): masked keys contribute exactly
zero to the softmax (the reference's exp(-1e6 - max) underflows to 0.0), so
only k < valid_len needs computing. The valid (batch, q-half, k-chunk) space
is split into uniform work tiles of (128 q-rows x 512 keys); tiles are
distributed round-robin over the 8 cores (padded with zero-mask dummy tiles
to a multiple of 8, T = tiles-per-core is 1..4). Every core runs the same
SPMD graph over T tiles. Each tile emits the UNNORMALIZED partials
(sum_k p*V | sum_k p) as a (128, 65) block; the host sums partials of the
same (batch, q-half) across tiles and divides - the cross-shard softmax
renormalization. No max-subtraction is needed: |scores| <= ||wv||_1 (~5),
so exp never overflows, and the missing shift cancels in the p/l ratio.
Math is exact up to rounding; bf16 matmul inputs with fp32 PSUM
accumulation give ~3e-3 relative error on the final output.

Per-tile device pipeline (q=128 -> 32 groups of 4, k=512):
  - partitions carry (j, h) = (q mod 4, h) -> 4*32 = 128 lanes
  - kh4 (kh replicated 4x over partition groups) via one col-tiled matmul
    set into a 1-bank psum tile, narrowed to bf16 in SBUF
  - qh4[(j,h), g] = qh[4g+j, h] via 4 col-tiled matmuls
  - loop over q-group chunks (2,2,4,8,8,8): DVE adds the per-group bias
    (per-partition scalar), ScalarE runs one big in-place tanh per chunk,
    TensorE reduces over h with zero-padded (128, 32) stationary weights
    (M=32 supergroup col-tiling) accumulating scores in psum
  - P = exp(scores) from psum; PE transposes P (4 tiles of 128x128);
    DVE multiplies by the 0/1 mask column during the psum->sbuf copy;
    accumulate [V | 1] matmuls into the (128, 65) partial output
Successive tiles pipeline: tile t+1's tanh stream runs while tile t's
softmax tail finishes, so only the last tile's tail is exposed.
"""

import ml_dtypes
import numpy as np

import concourse.bacc as bacc
import concourse.tile as tile
from concourse import mybir
from concourse.bass_utils import run_bass_kernel_spmd

B, NQ, NK = 4, 256, 2048
QKD, H, VD = 64, 32, 64
NQS = 128          # q rows per tile
NG = NQS // 4      # 32 q-groups of 4
KC = 512           # keys per tile
KT = KC // 128     # 4 k-subtiles per tile
CHUNKS = [2, 2, 4, 8, 8, 8]
CHUNKS_LAST = [2, 2, 4, 8, 8, 4, 2, 1, 1]
F32 = mybir.dt.float32
BF16 = mybir.dt.bfloat16

_cache = {}


def _build_nc(T):
    """Build the SPMD graph processing T work tiles per core."""
    nc = bacc.Bacc("TRN2", debug=False, num_devices=8,
                   monotonic_sem_count=0, enable_asserts=False)

    # blob columns: [0:32]=wq, [32:64]=wk, [64:64+128T]=qTr per tile,
    # [64+128T:192+128T]=ident, then 4T mask columns
    BW = 192 + 132 * T
    d_kT = nc.declare_dram_parameter("kT", [QKD, KC * T], BF16, isOutput=False)
    d_blob = nc.declare_dram_parameter("blob", [128, BW], BF16, isOutput=False)
    d_wvb = nc.declare_dram_parameter("wvb", [128, NG * 32], BF16, isOutput=False)
    d_vaug = nc.declare_dram_parameter("vaug", [128, KT * 65 * T], BF16,
                                       isOutput=False)
    d_out = nc.declare_dram_parameter("out", [NQS, 65 * T], F32, isOutput=True)

    TANH = mybir.ActivationFunctionType.Tanh
    EXP = mybir.ActivationFunctionType.Exp

    with tile.TileContext(nc) as tc:
        with (
            tc.tile_pool(name="sb", bufs=1) as sb,
            tc.tile_pool(name="fpool", bufs=2) as fpool,
            tc.tile_pool(name="psA", bufs=1, space="PSUM") as psA,
            tc.tile_pool(name="psB", bufs=1, space="PSUM") as psB,
        ):
            kT_sb = sb.tile([QKD, KC * T], BF16, tag="kT")
            blob_sb = sb.tile([128, BW], BF16, tag="blob")
            wvb_sb = sb.tile([128, NG * 32], BF16, tag="wvb")
            vaug_sb = sb.tile([128, KT * 65 * T], BF16, tag="vaug")
            qh4_sb = sb.tile([128, NG * T], F32, tag="qh4")
            kh4bf_sb = sb.tile([128, KC * T], BF16, tag="kh4bf")
            wq_sb = blob_sb[0:QKD, 0:32]
            wk_sb = blob_sb[0:QKD, 32:64]
            qTr_all = blob_sb[0:QKD, 64:64 + 128 * T]
            ident_sb = blob_sb[:, 64 + 128 * T:192 + 128 * T]
            maskc_bf = blob_sb[:, 192 + 128 * T:BW]
            maskc_sb = sb.tile([128, 4 * T], F32, tag="maskf")
            P_sb = sb.tile([128, KC * T], BF16, tag="P")
            PT_sb = sb.tile([128, KC * T], BF16, tag="PT")
            out_sb = sb.tile([NQS, 65 * T], F32, tag="outsb")

            # split the early DMAs across engine queues
            nc.sync.dma_start(out=kT_sb[:, 0:KC], in_=d_kT[:, 0:KC])
            if T > 1:
                nc.scalar.dma_start(out=kT_sb[:, KC:], in_=d_kT[:, KC:])
            nc.gpsimd.dma_start(out=blob_sb[:], in_=d_blob[:])
            nc.vector.tensor_copy(maskc_sb[:], maskc_bf)

            qh4_ps = psB.tile([128, NG * T], F32, tag="sc0")
            for t in range(T):
                for j in range(4):
                    nc.tensor.matmul(
                        qh4_ps[32 * j:32 * (j + 1), t * NG:(t + 1) * NG],
                        lhsT=wq_sb,
                        rhs=qTr_all[:, t * 128 + j * 32:t * 128 + (j + 1) * 32],
                        start=True, stop=True,
                        tile_position=(0, 32 * j),
                    )
            nc.vector.tensor_copy(qh4_sb[:], qh4_ps[:])

            # per-tile kh4 psum (1 bank each) -> bf16 sbuf
            kh4c = [psA.tile([128, KC], F32, tag=f"kh{t}", name=f"kh4c{t}")
                    for t in range(T)]
            for t in range(T):
                for j in range(4):
                    nc.tensor.matmul(
                        kh4c[t][32 * j:32 * (j + 1), :],
                        lhsT=wk_sb,
                        rhs=kT_sb[:, t * KC:(t + 1) * KC],
                        start=True, stop=True,
                        tile_position=(0, 32 * j),
                    )
                # alternate cast engines so they pipeline
                cp = nc.scalar.copy if t % 2 == 0 else nc.vector.tensor_copy
                cp(kh4bf_sb[:, t * KC:(t + 1) * KC], kh4c[t][:])

            scores = [psB.tile([128, KC], F32, tag=f"sc{t}", name=f"sc{t}")
                      for t in range(T)]
            PTb = [psA.tile([128, KC], BF16, tag=f"kh{t}", name=f"PTb{t}")
                   for t in range(T)]
            av = [psB.tile([128, 65], F32, tag=f"sc{t}", name=f"av{t}")
                  for t in range(T)]

            def score_mm(t, gg, rhs):
                G = gg // 8
                nc.tensor.matmul(
                    scores[t][32 * G:32 * (G + 1), :],
                    lhsT=wvb_sb[:, gg * 32:(gg + 1) * 32],
                    rhs=rhs,
                    start=(gg % 8 == 0), stop=(gg % 8 == 7),
                    skip_group_check=True,
                    tile_position=(0, 32 * G),
                )

            for t in range(T):
                # ---- tanh + h-reduction over this tile's 512 keys ----
                g = 0
                if t == 0:
                    # bridge the cast+add startup latency: first two groups
                    # tanh straight from the kh4 psum with a per-group bias
                    for gg in range(2):
                        Fb = fpool.tile([128, KC], BF16, tag="Fs1",
                                        bufs=4, name=f"Fb_{gg}")
                        nc.scalar.activation(
                            Fb[:], kh4c[0][:], TANH,
                            bias=qh4_sb[:, gg:gg + 1], scale=1.0,
                        )
                        if gg == 0:
                            nc.scalar.dma_start(out=wvb_sb[:], in_=d_wvb[:])
                            nc.gpsimd.dma_start(out=vaug_sb[:], in_=d_vaug[:])
                        score_mm(0, gg, Fb[:])
                    g = 2
                chunks = CHUNKS_LAST if t == T - 1 else CHUNKS
                if t == 0:
                    chunks = [2, 4, 8, 8, 8] if T > 1 else [4, 8, 8, 4, 2, 2, 1, 1]
                for nch in chunks:
                    Fs = fpool.tile([128, nch * KC], BF16, tag=f"Fs{nch}",
                                    bufs={1: 4, 2: 4, 4: 3, 8: 3}[nch],
                                    name=f"Fs_{t}_{g}")
                    for i in range(nch):
                        nc.vector.tensor_scalar_add(
                            Fs[:, i * KC:(i + 1) * KC],
                            kh4bf_sb[:, t * KC:(t + 1) * KC],
                            qh4_sb[:, t * NG + g + i:t * NG + g + i + 1],
                        )
                    nc.scalar.activation(Fs[:], Fs[:], TANH)
                    for i in range(nch):
                        score_mm(t, g + i, Fs[:, i * KC:(i + 1) * KC])
                    g += nch

                # ---- softmax numerator + masked AV partials ----
                nc.scalar.activation(
                    P_sb[:, t * KC:(t + 1) * KC], scores[t][:], EXP)
                for s in range(KT):
                    pcol = t * KC + s * 128
                    pt = PTb[t][:, s * 128:(s + 1) * 128]
                    nc.tensor.transpose(
                        pt, P_sb[:, pcol:pcol + 128], ident_sb)
                    nc.vector.tensor_scalar_mul(
                        PT_sb[:, pcol:pcol + 128], pt,
                        maskc_sb[:, t * KT + s:t * KT + s + 1],
                    )
                    nc.tensor.matmul(
                        av[t][:],
                        lhsT=PT_sb[:, pcol:pcol + 128],
                        rhs=vaug_sb[:, (t * KT + s) * 65:(t * KT + s + 1) * 65],
                        start=(s == 0), stop=(s == KT - 1),
                    )
                nc.vector.tensor_copy(
                    out_sb[:, t * 65:(t + 1) * 65], av[t][:])

            nc.sync.dma_start(out=d_out[:], in_=out_sb[:])

    nc.compile()
    return nc


def _host_shards(queries, keys, values, valid_lens, Wq, Wk, wv):
    """Build the balanced valid-key tile assignment and per-core inputs.
    Host work is layout/marshaling only; all tensor FLOPs run on device."""
    f32 = np.float32
    bf16 = ml_dtypes.bfloat16
    queries = np.asarray(queries, f32)
    keys = np.asarray(keys, f32)
    values = np.asarray(values, f32)
    valid_lens = np.asarray(valid_lens)
    Wq = np.asarray(Wq, f32)
    Wk = np.asarray(Wk, f32)
    wv = np.asarray(wv, f32)

    # work tiles: (batch, q-half, k-chunk) over the valid key range
    tiles = []
    for b in range(B):
        nk_chunks = max(1, int(np.ceil(int(valid_lens[b]) / KC)))
        for half in range(NQ // NQS):
            for kc in range(nk_chunks):
                tiles.append((b, half, kc))
    while len(tiles) % 8 != 0:
        tiles.append(None)                     # zero-mask dummy
    T = len(tiles) // 8

    # zero-padded stationary weights (M=32 supergroup col-tiling)
    wvb = np.zeros((128, NG * 32), f32)
    for g in range(NG):
        for j in range(4):
            wvb[j * 32:(j + 1) * 32, g * 32 + 4 * (g % 8) + j] = wv

    BW = 192 + 132 * T
    blob_base = np.zeros((128, BW), f32)
    blob_base[0:QKD, 0:32] = Wq
    blob_base[0:QKD, 32:64] = Wk
    blob_base[:, 64 + 128 * T:192 + 128 * T] = np.eye(128, dtype=f32)
    shared = {"wvb": wvb.astype(bf16)}

    assign = [tiles[c::8] for c in range(8)]   # round-robin -> balanced
    in_maps = []
    for core in range(8):
        kT = np.zeros((QKD, KC * T), f32)
        vaug = np.zeros((128, KT * 65 * T), f32)
        blob = blob_base.copy()
        for t, tl in enumerate(assign[core]):
            if tl is None:
                continue
            b, half, kc = tl
            qs = queries[b, half * NQS:(half + 1) * NQS]      # (128, 64)
            qTr = np.ascontiguousarray(
                qs.T.reshape(QKD, NG, 4).transpose(0, 2, 1)).reshape(QKD, NQS)
            blob[0:QKD, 64 + 128 * t:64 + 128 * (t + 1)] = qTr
            kT[:, t * KC:(t + 1) * KC] = keys[b, kc * KC:(kc + 1) * KC].T
            v = values[b, kc * KC:(kc + 1) * KC].reshape(KT, 128, VD)
            va = np.concatenate([v, np.ones((KT, 128, 1), f32)], axis=2)
            vaug[:, t * KT * 65:(t + 1) * KT * 65] = (
                va.transpose(1, 0, 2).reshape(128, KT * 65))
            kmask = (np.arange(kc * KC, (kc + 1) * KC)
                     < int(valid_lens[b])).astype(f32)
            blob[:, 192 + 128 * T + 4 * t:192 + 128 * T + 4 * (t + 1)] = (
                kmask.reshape(KT, 128).T)
        in_maps.append({
            "kT": np.ascontiguousarray(kT).astype(bf16),
            "blob": blob.astype(bf16),
            "vaug": np.ascontiguousarray(vaug).astype(bf16),
            **shared,
        })
    return T, assign, in_maps


def kernel(queries, keys, values, valid_lens, Wq, Wk, wv, _trace=False):
    T, assign, in_maps = _host_shards(
        queries, keys, values, valid_lens, Wq, Wk, wv)
    if ("nc", T) not in _cache:
        _cache[("nc", T)] = _build_nc(T)
    nc = _cache[("nc", T)]

    res = None
    for attempt in range(3):
        try:
            res = run_bass_kernel_spmd(
                nc, in_maps, core_ids=list(range(8)), trace=_trace
            )
            break
        except Exception:
            if attempt == 2:
                raise
            if attempt == 1:
                _cache.pop(("nc", T), None)
                _cache[("nc", T)] = nc = _build_nc(T)
    _cache["last_result"] = res

    # cross-shard softmax renormalization (the unshard/combine step)
    acc = np.zeros((B, NQ // NQS, NQS, VD + 1), np.float64)
    for core in range(8):
        part = res.results[core]["out"]        # (128, 65*T)
        for t, tl in enumerate(assign[core]):
            if tl is None:
                continue
            b, half, _ = tl
            acc[b, half] += part[:, t * 65:(t + 1) * 65].astype(np.float64)
    out = acc[..., :VD] / acc[..., VD:VD + 1]
    return np.ascontiguousarray(
        out.reshape(B, NQ, VD).astype(np.float32))
